# revision 14
# baseline (speedup 1.0000x reference)
"""MoE HyperNet linear layer on 8 Trainium2 NeuronCores.

Reference computation (B=4096, I=O=1024, C=128, E=8):
    h      = relu(cond @ g_w1 + g_b1)                # [B, 4E]
    gating = softmax(h @ g_w2 + g_b2, axis=1)        # [B, E]
    out    = einsum('be,beo->bo', gating,
                    einsum('bi,eio->beo', x, W)) + gating @ expert_biases

Strategy: data-parallel shard B across the 8 cores (512 rows each),
replicate all weights, and fold the gate into the activations:

    out[b,o] = sum_e sum_i (g[b,e]*x[b,i]) W_e[i,o] + (gating @ biases)[b,o]

so the whole MoE collapses into ONE K=8192 GEMM per core that the PE
accumulates entirely in PSUM — no per-expert combine pass.

Per core:
  - x/cond shards are passed in pre-transposed ([feature, batch]) — a
    host-side layout choice during sharding, like the [E*I, O] W reshape.
  - gating MLP runs transposed ([4E,512] -> [8,512]); softmax over the 8
    experts via exp + an all-ones K=8 matmul + reciprocal (no max-shift:
    logits here are O(1)).
  - gate rows are broadcast to 128 partitions with one-hot selector
    matmuls (gb_all), then xtg_e = xT * g_e (DVE, output rounded to
    float32r) feeds the PE as the stationary operand.
  - main GEMM: out[bc][b,o] += xtg_e[ic,bc].T @ W_e[ic,oh] accumulated
    over all (e, ic) in 4 persistent [128,1024] PSUM tiles (8 banks);
    the expert-bias term (gT.T @ biases) is appended to the same
    accumulation chain before stop.
  - output is produced in natural [b, o] orientation; the host just
    concatenates core shards.

Big-GEMM operands are float32r (fast fp32 PE mode, ~1 cycle/row at
N>=256 vs 4 for plain fp32, rel.err ~1e-4): W/sel/eb are rounded by
casting gpsimd DMAs, xtg/gT by DVE output dtype.

Any instruction here can carry only ONE sync wait (walrus limit), so a
post-pass splits extra waits onto same-engine NoOps (_split_waits).
"""

import sys

if "/opt/trn_rl_repo" not in sys.path:
    sys.path.insert(0, "/opt/trn_rl_repo")

import numpy as np

import bass_rust
import concourse.bass as bass
import concourse.mybir as mybir
import concourse.tile as tile
from concourse.bass_utils import run_bass_kernel_spmd


def _split_waits(nc, max_waits=1):
    """Hoist all-but-one sync wait of each instruction onto same-engine
    NoOps inserted directly before it. This walrus build rejects any TPB
    instruction carrying more than one wait ("Too many sync wait
    commands"); engines are in-order so the split preserves semantics."""
    for bb in nc.m.functions[0].blocks:
        out = []
        for i in list(bb.instructions):
            si = i.sync_info
            waits = list(si.on_wait) if si else []
            if len(waits) > max_waits:
                for k, w in enumerate(waits[:-max_waits]):
                    nop = mybir.InstNoOp(
                        name=f"{i.name}-waitsplit{k}", ins=[], outs=[])
                    nop.engine = i.engine
                    nop.sync_info = bass_rust.SyncInfo(on_wait=[w], on_update=[])
                    out.append(nop)
                i.sync_info = bass_rust.SyncInfo(
                    on_wait=waits[-max_waits:], on_update=list(si.on_update))
            out.append(i)
        bb.instructions = out

B, I, O, C, E = 4096, 1024, 1024, 128, 8
N_CORES = 8
BS = B // N_CORES          # 512 batch rows per core
NB = BS // 128             # 4 batch chunks of 128
NI = I // 128              # 8 contraction chunks
NO2 = 2                    # two N=512 halves of O
H = 4 * E                  # 32 gating hidden

_cache = {}


def _build_nc():
    dt = mybir.dt
    f32, f32r = dt.float32, dt.float32r

    nc = bass.Bass("TRN2", target_bir_lowering=False, debug=False,
                   num_devices=N_CORES)

    xT_d = nc.dram_tensor("xT_sh", [I, BS], f32, kind="ExternalInput").ap()
    condT_d = nc.dram_tensor("condT_sh", [C, BS], f32, kind="ExternalInput").ap()
    # raw fp32 bits, declared f32r: HWDGE same-dtype DMAs, PE reads f32r
    w_d = nc.dram_tensor("w", [E * I, O], f32r, kind="ExternalInput").ap()
    eb_d = nc.dram_tensor("eb", [E, O], f32, kind="ExternalInput").ap()
    gpack_d = nc.dram_tensor("gpack", [128, 50], f32, kind="ExternalInput").ap()
    sel_d = nc.dram_tensor("sel", [E, E * 128], f32, kind="ExternalInput").ap()
    out_d = nc.dram_tensor("out_sh", [BS, O], f32, kind="ExternalOutput").ap()

    with tile.TileContext(nc) as tc:
        with (
            tc.tile_pool(name="consts", bufs=1) as consts,
            tc.tile_pool(name="xin", bufs=2) as xin,
            tc.tile_pool(name="stage", bufs=1) as stage,
            tc.tile_pool(name="wpool", bufs=2) as wpool,
            tc.tile_pool(name="xtgp", bufs=2) as xtgp,
            tc.tile_pool(name="outp", bufs=2) as outp,
        ):
            # ---- constants: one packed DMA for the whole gating MLP ----
            gpack = consts.tile([128, 50], f32, tag="gpack")
            nc.scalar.dma_start(gpack[:], gpack_d)
            gw1 = gpack[:, 0:H]            # [128, 32]
            gb1 = gpack[0:H, H:H + 1]      # [32, 1]
            gw2 = gpack[0:H, 33:33 + E]    # [32, 8]
            gb2 = gpack[0:E, 41:42]        # [8, 1]
            ones8 = gpack[0:E, 42:50]      # [8, 8]
            sel_r = consts.tile([E, E * 128], f32r, tag="sel_r")
            eb_r = consts.tile([E, O], f32r, tag="eb_r")

            xT = stage.tile([128, NI * BS], f32, tag="xT")
            condT = stage.tile([C, BS], f32, tag="condT")
            gb_all = stage.tile([128, E * BS], f32, tag="gb_all")
            gT_r = stage.tile([E, BS], f32r, tag="gT_r")

            with (
                tc.tile_pool(name="ps_g", bufs=2, space="PSUM") as ps_g,
            ):
                # ---- pre-transposed cond / x straight into SBUF ----
                nc.sync.dma_start(condT[:], condT_d)
                nc.gpsimd.dma_start(sel_r[:], sel_d)
                nc.gpsimd.dma_start(eb_r[:], eb_d)

                def dma_w(wt, e):
                    # wt[p, ic*O + o] = W[e*I + ic*128 + p, o]; one half per
                    # HWDGE ring (sync + scalar run in parallel)
                    for h2, eng in ((0, nc.sync), (1, nc.scalar)):
                        icn = NI // 2
                        rows = w_d[e * I + h2 * icn * 128:
                                   e * I + (h2 + 1) * icn * 128, :]
                        eng.dma_start(
                            wt[:, h2 * icn * O:(h2 + 1) * icn * O]
                            .rearrange("p (ic o) -> p ic o", ic=icn),
                            rows.rearrange("(ic p) o -> p ic o", p=128))

                # first expert's weights ahead of xT in the ring FIFOs
                wt0 = wpool.tile([128, NI * O], f32r, tag="w")
                dma_w(wt0, 0)

                # xT[p, ic*BS + b] = x[b, ic*128 + p]; one DMA per half
                xT3 = xT[:].rearrange("p (ic b) -> p ic b", ic=NI)
                xs3 = xT_d.rearrange("(ic p) b -> p ic b", p=128)
                for h2, eng in ((0, nc.sync), (1, nc.scalar)):
                    eng.dma_start(xT3[:, h2 * (NI // 2):(h2 + 1) * (NI // 2), :],
                                  xs3[:, h2 * (NI // 2):(h2 + 1) * (NI // 2), :])

                # ---- gating ----
                ph = ps_g.tile([128, BS], f32, tag="ps_g")
                nc.tensor.matmul(ph[0:H, :], gw1, condT[:],
                                 start=True, stop=True)
                hT = stage.tile([H, BS], f32, tag="hT")
                nc.vector.tensor_scalar_add(hT[:], ph[0:H, :], gb1)
                nc.vector.tensor_relu(hT[:], hT[:])
                pz = ps_g.tile([128, BS], f32, tag="ps_g")
                nc.tensor.matmul(pz[0:E, :], gw2, hT[:],
                                 start=True, stop=True)
                ezT = stage.tile([E, BS], f32, tag="ezT")
                nc.scalar.activation(ezT[:], pz[0:E, :],
                                     mybir.ActivationFunctionType.Exp,
                                     bias=gb2, scale=1.0)
                pden = ps_g.tile([128, BS], f32, tag="ps_g")
                nc.tensor.matmul(pden[0:E, :], ones8, ezT[:],
                                 start=True, stop=True)
                rden = stage.tile([E, BS], f32, tag="rden")
                nc.vector.reciprocal(rden[:], pden[0:E, :])
                # normalized gates, rounded to f32r (feeds bias + gb matmuls)
                nc.vector.tensor_mul(gT_r[:], ezT[:], rden[:])

                # gate rows broadcast to 128 partitions (fp32)
                for e in range(E):
                    pgb = ps_g.tile([128, BS], f32, tag="ps_g")
                    nc.tensor.matmul(pgb[:], sel_r[:, e * 128:(e + 1) * 128],
                                     gT_r[:], start=True, stop=True)
                    nc.vector.tensor_copy(gb_all[:, e * BS:(e + 1) * BS], pgb[:])

            # ---- main GEMM: 4 persistent [128,1024] PSUM accumulators ----
            with tc.tile_pool(name="ps_main", bufs=1, space="PSUM") as ps_main:
                pouts = []
                for bc in range(NB):
                    po = ps_main.tile([128, O], f32, tag=f"po{bc}")
                    pouts.append(po)
                for e in range(E):
                    if e == 0:
                        wt = wt0
                    else:
                        wt = wpool.tile([128, NI * O], f32r, tag="w")
                        dma_w(wt, e)
                    # xtg_e = xT * g_e  (fp32 inputs, f32r output)
                    xtg = xtgp.tile([128, NI * BS], f32r, tag="xtg")
                    for ic in range(NI):
                        nc.vector.tensor_mul(
                            xtg[:, ic * BS:(ic + 1) * BS],
                            xT[:, ic * BS:(ic + 1) * BS],
                            gb_all[:, e * BS:(e + 1) * BS])
                    if e < E - 1:
                        for ic in range(NI):
                            for bc in range(NB):
                                lhsT = xtg[:, ic * BS + bc * 128:
                                           ic * BS + (bc + 1) * 128]
                                for oh in range(NO2):
                                    nc.tensor.matmul(
                                        pouts[bc][:, oh * 512:(oh + 1) * 512],
                                        lhsT,
                                        wt[:, ic * O + oh * 512:
                                           ic * O + (oh + 1) * 512],
                                        start=(e == 0 and ic == 0), stop=False)
                    else:
                        # last expert bc-major: finish each batch chunk (bias
                        # + copy + store) while the others still compute
                        for bc in range(NB):
                            for ic in range(NI):
                                lhsT = xtg[:, ic * BS + bc * 128:
                                           ic * BS + (bc + 1) * 128]
                                for oh in range(NO2):
                                    nc.tensor.matmul(
                                        pouts[bc][:, oh * 512:(oh + 1) * 512],
                                        lhsT,
                                        wt[:, ic * O + oh * 512:
                                           ic * O + (oh + 1) * 512],
                                        start=False, stop=False)
                            for oh in range(NO2):
                                nc.tensor.matmul(
                                    pouts[bc][:, oh * 512:(oh + 1) * 512],
                                    gT_r[:, bc * 128:(bc + 1) * 128],
                                    eb_r[:, oh * 512:(oh + 1) * 512],
                                    start=False, stop=True)
                            osb = outp.tile([128, O], f32, tag="osb")
                            nc.vector.tensor_copy(osb[:], pouts[bc][:])
                            nc.sync.dma_start(
                                out_d[bc * 128:(bc + 1) * 128, :], osb[:])

    _split_waits(nc)
    return nc


def _get_nc():
    if "nc" not in _cache:
        _cache["nc"] = _build_nc()
    return _cache["nc"]


def _make_in_maps(x, cond, expert_weights, expert_biases, g_w1, g_b1, g_w2, g_b2):
    w_flat = np.ascontiguousarray(
        np.asarray(expert_weights, dtype=np.float32).reshape(E * I, O))
    xT = np.asarray(x, dtype=np.float32).T    # [I, B]
    condT = np.asarray(cond, dtype=np.float32).T  # [C, B]
    sel = np.zeros((E, E * 128), dtype=np.float32)
    for e in range(E):
        sel[e, e * 128:(e + 1) * 128] = 1.0
    gpack = np.zeros((128, 50), dtype=np.float32)
    gpack[:, 0:H] = np.asarray(g_w1, dtype=np.float32)
    gpack[0:H, H] = np.asarray(g_b1, dtype=np.float32)
    gpack[0:H, 33:33 + E] = np.asarray(g_w2, dtype=np.float32)
    gpack[0:E, 41] = np.asarray(g_b2, dtype=np.float32)
    gpack[0:E, 42:50] = 1.0
    common = {
        "w": w_flat,
        "eb": np.ascontiguousarray(np.asarray(expert_biases, dtype=np.float32)),
        "gpack": gpack,
        "sel": sel,
    }
    in_maps = []
    for c in range(N_CORES):
        m = dict(common)
        m["xT_sh"] = np.ascontiguousarray(xT[:, c * BS:(c + 1) * BS])
        m["condT_sh"] = np.ascontiguousarray(condT[:, c * BS:(c + 1) * BS])
        in_maps.append(m)
    return in_maps


def run(inputs, trace=False, **kw):
    """Build + run; returns (full_out [B, O] fp32, BassKernelResults)."""
    nc = _get_nc()
    in_maps = _make_in_maps(**inputs)
    res = run_bass_kernel_spmd(nc, in_maps, core_ids=list(range(N_CORES)),
                               trace=trace, **kw)
    out = np.concatenate([res.results[c]["out_sh"] for c in range(N_CORES)],
                         axis=0)
    return out, res


def kernel(**inputs):
    out, _ = run(inputs)
    return out
